# revision 84
# baseline (speedup 1.0000x reference)
"""Trainium2 Bass kernel for a ViT/Swin-style transformer block.

Strategy: data-parallel over batch (64 -> 8 per core), no collectives.
Feature-major on-device layout: activations [features(128-part, k-tiles),
tokens(free)].  Large GEMMs run in fp8e4m3 with DoubleRow perf mode
(256-deep contraction, 2x PE throughput) accumulating in fp32 PSUM;
attention score/AV matmuls are bf16.

The program is issued as two software-pipelined streams of per-unit work
items (unit = 2 batch elements = 394 token columns) so that the DVE/ACT
heavy attention overlaps the PE-heavy QKV GEMMs, and the DVE-heavy
proj+LN2 overlaps the PE/ACT-heavy MLP:
    LN1(u)+QKV(u)  ||  ATTN(u-1)
    PROJ+LN2(u)    ||  MLP(u-1)

Quantization scales (folded, compensated in the psum->sbuf cast):
  weights  *2^Sw  (per-tensor, host-chosen to fill e4m3 normal range)
  h1/h2    *16    (LN outputs; folded into the Sqrt(var) stage for free)
  aoT      *32    (attention out; folded into the softmax reciprocal)

Softmax: scores + rpb (PE identity-matmul accumulate) -> exp (ACT) ->
denominator via ones-column matmul (PE) -> reciprocal (DVE) ->
partition_broadcast (Pool) -> folded into the AV-psum -> aoT fp8 cast.
"""

import numpy as np
import ml_dtypes
from contextlib import ExitStack

import concourse.bacc as bacc
import concourse.bass as bass
import concourse.mybir as mybir
import concourse.tile as tile
from concourse.bass_utils import run_bass_kernel_spmd

bf16 = ml_dtypes.bfloat16
dt = mybir.dt
f8 = mybir.dt.np(dt.float8e4)
AF = mybir.ActivationFunctionType
ALU = mybir.AluOpType
PM = mybir.MatmulPerfMode

# ---- problem dims (hardcoded) ----
B, N, D, H, DH, HID = 64, 197, 768, 12, 64, 3072
NCORES = 8
BPC = B // NCORES          # 8 batch elements per core
T = BPC * N                # 1576 token-columns per core
KT = D // 128              # 6 feature k-tiles
KP = KT // 2               # 3 DoubleRow k-pairs
HT = HID // 128            # 24 hidden tiles
HP = HT // 2               # 12 DoubleRow hidden pairs
NU = 4                     # units (2 batch elements each)
CH = T // NU               # 394 token-cols per unit
MT = 2                     # m-tiles per batch element (128 + 69)
MSZ = [128, N - 128]       # [128, 69]
EPS = 1e-5
HSC = 16.0                 # LN-output fp8 scale
ASC = 32.0                 # attention-output fp8 scale

_NC_CACHE = {}


def _merge(*streams):
    """Proportional round-robin: issue items from all streams interleaved
    so every stream finishes at the same fractional position."""
    counts = [len(s) for s in streams]
    mx = max(counts) if counts else 0
    for t in range(mx):
        for s, cnt in zip(streams, counts):
            for item in s[t * cnt // mx:(t + 1) * cnt // mx]:
                item()


def _build_nc(flags):
    if flags in _NC_CACHE:
        return _NC_CACHE[flags]
    vb_any, pb_any, b2_any = flags
    nc = bacc.Bacc(None, target_bir_lowering=False)

    # ---- DRAM I/O ----
    d_xT = nc.dram_tensor("xT", [D, T], dt.bfloat16, kind="ExternalInput")
    d_wqkv = nc.dram_tensor("wqkvT", [D, 3 * D], dt.float8e4, kind="ExternalInput")
    d_wp = nc.dram_tensor("wpT", [D, D], dt.float8e4, kind="ExternalInput")
    d_w1 = nc.dram_tensor("w1T", [D, HID], dt.float8e4, kind="ExternalInput")
    d_w2 = nc.dram_tensor("w2T", [HID, D], dt.float8e4, kind="ExternalInput")
    d_qb = nc.dram_tensor("qb", [128, KT], dt.float32, kind="ExternalInput")
    d_kb = nc.dram_tensor("kb", [128, KT], dt.float32, kind="ExternalInput")
    d_vb = nc.dram_tensor("vb", [1, D], dt.bfloat16, kind="ExternalInput")
    d_pb = nc.dram_tensor("pb", [128, KT], dt.float32, kind="ExternalInput")
    d_b1 = nc.dram_tensor("b1", [128, HT], dt.float32, kind="ExternalInput")
    d_b2 = nc.dram_tensor("b2", [128, KT], dt.float32, kind="ExternalInput")
    d_sc = nc.dram_tensor("scales", [128, 8], dt.float32, kind="ExternalInput")
    d_eye = nc.dram_tensor("eye", [128, 128], dt.float8e4, kind="ExternalInput")
    d_rpb = nc.dram_tensor("rpbT", [128, H, MT * N], dt.float8e4, kind="ExternalInput")
    d_yT = nc.dram_tensor("yT", [D, T], dt.float32, kind="ExternalOutput")

    with ExitStack() as ctx:
        tc = ctx.enter_context(tile.TileContext(nc))

        p_const = tc.alloc_tile_pool(name="const", bufs=1)
        p_rows = tc.alloc_tile_pool(name="prows", bufs=2)
        p_big = tc.alloc_tile_pool(name="pbig", bufs=1)

        # constants
        ones_mu = p_const.tile([128, 1], dt.bfloat16)      # 1/768 column
        ones_r = p_const.tile([1, 128], dt.bfloat16)       # 1.0 row
        eps_t = p_const.tile([1, 1], dt.float32)
        nc.vector.memset(ones_mu[:], 1.0 / D)
        nc.vector.memset(ones_r[:], 1.0)
        nc.vector.memset(eps_t[:], EPS / (HSC * HSC))
        t_eye = p_const.tile([128, 128], dt.float8e4)
        nc.sync.dma_start(t_eye[:], d_eye[:])
        ones_w = p_const.tile([1, 512], dt.bfloat16)
        nc.vector.memset(ones_w[:], 1.0)
        t_qb = p_const.tile([128, KT], dt.float32)
        t_kb = p_const.tile([128, KT], dt.float32)
        t_vb = p_const.tile([1, D], dt.bfloat16)
        t_pb = p_const.tile([128, KT], dt.float32)
        t_b1 = p_const.tile([128, HT], dt.float32)
        t_b2 = p_const.tile([128, KT], dt.float32)
        t_sc = p_const.tile([128, 8], dt.float32)
        for t_, d_ in [(t_qb, d_qb), (t_kb, d_kb), (t_vb, d_vb),
                       (t_pb, d_pb), (t_b1, d_b1), (t_b2, d_b2), (t_sc, d_sc)]:
            nc.sync.dma_start(t_[:], d_[:])
        sq, sk, sv, sp, s1, s2 = (t_sc[:, i:i + 1] for i in range(6))

        # ---- long-lived tiles ----
        # x lives in bf16: LN inputs read it directly and the residual
        # updates round through bf16 (error ~0.4% of |x|, within tolerance)
        xT = p_big.tile([128, KT, T], dt.bfloat16, tag="tg_x")
        xTr = d_xT.rearrange("(k p) t -> p k t", p=128)
        for c in range(NU):   # chunk-major so LN1(0) starts after 1st DMA
            cs = bass.ts(c, CH)
            nc.gpsimd.dma_start(xT[:, :, cs], xTr[:, :, cs])

        wqkv = p_big.tile([128, KP, 2, 3 * D], dt.float8e4, tag="tg_wqkv")
        nc.sync.dma_start(wqkv[:], d_wqkv.rearrange("(k p) m -> p k m", p=128)
                          .rearrange("p (a b) m -> p a b m", b=2))
        wp = p_big.tile([128, KP, 2, D], dt.float8e4, tag="tg_wp")
        nc.sync.dma_start(wp[:], d_wp.rearrange("(k p) m -> p k m", p=128)
                          .rearrange("p (a b) m -> p a b m", b=2))
        rpb = p_big.tile([128, H, MT * N], dt.float8e4, tag="tg_rpb")
        nc.sync.dma_start(rpb[:], d_rpb[:])

        w1 = p_big.tile([128, KP, 2, HID], dt.float8e4, tag="tg_w1")
        nc.sync.dma_start(w1[:], d_w1.rearrange("(k p) m -> p k m", p=128)
                          .rearrange("p (a b) m -> p a b m", b=2))

        h1 = p_big.tile([128, KP, 2, T], dt.float8e4, tag="tg_h")
        qT = p_big.tile([128, KT, T], dt.float8e4, tag="tg_q")
        kTt = p_big.tile([128, KT, T], dt.float8e4, tag="tg_k")
        # token-major V: vtokT[token, b, mt, ktile, feat] (feat = 2 heads x 64)
        vtokT = p_big.tile([128, BPC, MT, KT, 128], dt.bfloat16, tag="tg_v")
        aoT = p_big.tile([128, KP, 2, T], dt.float8e4, tag="tg_ao")

        p_atmp = tc.alloc_tile_pool(name="patmp", bufs=1)
        p_vt = tc.alloc_tile_pool(name="pvt", bufs=1)

        # ============ LayerNorm per-chunk items (feature-major) ============
        # dst_f8[:, kp, e, cs] = HSC * (x - mu)/sigma
        def ln_items(c, src_bf, dst_f8, psum_pool, stat_tag):
            cs = bass.ts(c, CH)
            st = {}

            def stats():
                x2 = p_atmp.tile([128, KT, CH], dt.bfloat16, tag="x2", bufs=2)
                nc.vector.tensor_mul(x2[:], src_bf[:, :, cs], src_bf[:, :, cs])
                mu_ps = psum_pool.tile([1, CH], dt.float32, tag=stat_tag, bufs=2)
                ms_ps = psum_pool.tile([1, CH], dt.float32, tag=stat_tag, bufs=2)
                for k in range(KT):
                    nc.tensor.matmul(mu_ps[:], ones_mu[:], src_bf[:, k, cs],
                                     start=(k == 0), stop=(k == KT - 1))
                for k in range(KT):
                    nc.tensor.matmul(ms_ps[:], ones_mu[:], x2[:, k, :],
                                     start=(k == 0), stop=(k == KT - 1))
                st["mu"], st["ms"] = mu_ps, ms_ps

            def rows():
                mu_ps, ms_ps = st["mu"], st["ms"]
                musq = p_rows.tile([1, CH], dt.float32, tag="musq")
                nc.scalar.square(musq[:], mu_ps[:])
                var = p_rows.tile([1, CH], dt.float32, tag="var")
                nc.vector.tensor_sub(var[:], ms_ps[:], musq[:])
                std = p_rows.tile([1, CH], dt.float32, tag="std")
                nc.scalar.activation(std[:], var[:], AF.Sqrt,
                                     bias=eps_t[0:1, 0:1], scale=1.0 / (HSC * HSC))
                a_f = p_rows.tile([1, CH], dt.float32, tag="af")
                nc.vector.reciprocal_approx_fast(a_f[:], std[:])     # HSC/sigma
                a_b = p_rows.tile([1, CH], dt.bfloat16, tag="ab")
                nc.gpsimd.tensor_copy(a_b[:], a_f[:])
                b_b = p_rows.tile([1, CH], dt.bfloat16, tag="bb")
                with nc.allow_low_precision(reason="LN shift row bf16"):
                    nc.vector.scalar_tensor_tensor(b_b[:], mu_ps[:], -1.0, a_f[:],
                                                   op0=ALU.mult, op1=ALU.mult)
                bca = p_atmp.tile([128, CH], dt.bfloat16, tag="bca", bufs=2)
                nc.gpsimd.partition_broadcast(bca[:], a_b[:])
                bcb = p_atmp.tile([128, CH], dt.bfloat16, tag="bcb", bufs=2)
                nc.gpsimd.partition_broadcast(bcb[:], b_b[:])
                st["bca"], st["bcb"] = bca, bcb

            def apply(k0):
                for k in range(k0, k0 + 3):
                    tmp = p_atmp.tile([128, CH], dt.bfloat16, tag="ntmp", bufs=3)
                    nc.vector.tensor_mul(tmp[:], src_bf[:, k, cs], st["bca"][:])
                    nc.gpsimd.tensor_add(dst_f8[:, k // 2, k % 2, cs], tmp[:],
                                         st["bcb"][:])

            return [stats, rows, lambda: apply(0), lambda: apply(3)]

        # ============ QKV per-unit items ============
        def qkv_items(u, psum_pool):
            cs = bass.ts(u, CH)
            items = []

            def qk_tile(d_i, is_q):
                def f():
                    off = 0 if is_q else D
                    ps = psum_pool.tile([128, CH], dt.float32, tag="mm", bufs=3)
                    for kp in range(KP):
                        nc.tensor.matmul(ps[:], wqkv[:, kp, :, off + d_i * 128:
                                                     off + (d_i + 1) * 128],
                                         h1[:, kp, :, cs], start=(kp == 0),
                                         stop=(kp == KP - 1), perf_mode=PM.DoubleRow)
                    if is_q:
                        nc.scalar.activation(qT[:, d_i, cs], ps[:], AF.Identity,
                                             bias=t_qb[:, d_i:d_i + 1], scale=sq)
                    else:
                        nc.vector.tensor_scalar(kTt[:, d_i, cs], ps[:], sk,
                                                t_kb[:, d_i:d_i + 1],
                                                op0=ALU.mult, op1=ALU.add)
                return f

            def v_tile(d_i, vT):
                def f():
                    pv = psum_pool.tile([128, CH], dt.float32, tag="mm", bufs=3)
                    if vb_any:
                        nc.tensor.matmul(pv[:], t_vb[:, d_i * 128:(d_i + 1) * 128],
                                         ones_w[0:1, 0:CH], start=True, stop=False,
                                         skip_group_check=True)
                    for kp in range(KP):
                        nc.tensor.matmul(pv[:], wqkv[:, kp, :,
                                                     2 * D + d_i * 128:
                                                     2 * D + (d_i + 1) * 128],
                                         h1[:, kp, :, cs],
                                         start=(kp == 0 and not vb_any),
                                         stop=(kp == KP - 1), perf_mode=PM.DoubleRow,
                                         skip_group_check=vb_any)
                    if d_i % 2 == 0:
                        nc.scalar.activation(vT[:, d_i, 0:CH], pv[:], AF.Identity,
                                             scale=sv)
                    else:
                        nc.vector.tensor_scalar(vT[:, d_i, 0:CH], pv[:], sv, 0.0,
                                                op0=ALU.mult, op1=ALU.add)
                    # transpose this k-tile for both (b, mt) of the unit
                    for bl in range(2):
                        for mt in range(MT):
                            off = bl * N + mt * 128
                            nc.sync.dma_start_transpose(
                                vtokT[:, 2 * u + bl, mt, d_i, :],
                                vT[:, d_i, off:off + 128])
                return f

            vT = p_vt.tile([128, KT, 512], dt.bfloat16, tag="vt", bufs=2,
                           name=f"vT_{u}")
            for d_i in range(KT):
                items.append(qk_tile(d_i, True))
                items.append(qk_tile(d_i, False))
                items.append(v_tile(d_i, vT))
            return items

        # ============ attention per-unit items ============
        PAIRS = [(b, j) for b in range(BPC) for j in range(H // 2)]
        GRP = 4
        GROUPS = [PAIRS[g * GRP:(g + 1) * GRP] for g in range(len(PAIRS) // GRP)]
        astate = {}

        def g_scores(g, psum_pool):
            for p in GROUPS[g]:
                b, j = p
                ts_n = slice(b * N, (b + 1) * N)
                p_bf = p_aw.tile([128, 2, MT * N], dt.bfloat16, tag="pbf", bufs=7,
                                 name=f"pbf_{b}_{j}")
                for e in range(2):
                    hp = e * 64
                    sc = psum_pool.tile([128, MT * N], dt.float32, tag="sc", bufs=2,
                                        name=f"sc_{b}_{j}_{e}")
                    for mt in range(MT):
                        msz = MSZ[mt]
                        m0 = b * N + mt * 128
                        nc.tensor.matmul(sc[0:msz, bass.ts(mt, N)],
                                         kTt[hp:hp + 64, j, m0:m0 + msz],
                                         qT[hp:hp + 64, j, ts_n],
                                         start=True, stop=False,
                                         skip_group_check=True)
                    nc.tensor.matmul(sc[:], t_eye[:], rpb[:, 2 * j + e, :],
                                     start=False, stop=True, skip_group_check=True)
                    nc.scalar.activation(p_bf[:, e, :], sc[:], AF.Exp,
                                         scale=1.0 / (HSC * HSC))
                astate[p] = {"p_bf": p_bf}

        def g_denom(g, psum_pool):
            for p in GROUPS[g]:
                b, j = p
                pb = astate[p]["p_bf"]
                dn = psum_pool.tile([1, 2, N], dt.float32, tag="dn", bufs=1,
                                    name=f"dn_{b}_{j}")
                for mt in range(MT):
                    msz = MSZ[mt]
                    nc.tensor.matmul(dn[:], ones_mu[0:msz, 0:1],
                                     pb[0:msz, :, mt * N:(mt + 1) * N],
                                     start=(mt == 0), stop=(mt == MT - 1))
                rc = p_aw.tile([1, 2 * N], dt.float32, tag="rc", bufs=2,
                               name=f"rc_{b}_{j}")
                nc.vector.reciprocal_approx_fast(
                    rc[:], dn[:].rearrange("a e n -> a (e n)"))
                rcb = p_aw.tile([1, 2 * N], dt.bfloat16, tag="rcb", bufs=2,
                                name=f"rcb_{b}_{j}")
                with nc.allow_low_precision(reason="softmax recip bf16"):
                    nc.gpsimd.tensor_scalar(rcb[:], rc[:], ASC / D, 0.0,
                                            op0=ALU.mult, op1=ALU.add)
                bcs = p_aw.tile([128, 2, N], dt.bfloat16, tag="bcs", bufs=4,
                                name=f"bcs_{b}_{j}")
                nc.gpsimd.partition_broadcast(
                    bcs[:].rearrange("p e n -> p (e n)"), rcb[:])
                astate[p]["bcs"] = bcs

        def g_av(g, psum_pool):
            for p in GROUPS[g]:
                b, j = p
                st = astate[p]
                av = psum_pool.tile([64, 2, N], dt.float32, tag="av", bufs=2,
                                    name=f"av_{b}_{j}")
                for e in range(2):
                    for mt in range(MT):
                        nc.tensor.matmul(av[:, e, :],
                                         vtokT[0:MSZ[mt], b, mt, j,
                                               e * 64:(e + 1) * 64],
                                         st["p_bf"][0:MSZ[mt], e,
                                                    mt * N:(mt + 1) * N],
                                         start=(mt == 0), stop=(mt == MT - 1))
                for e in range(2):
                    dst = aoT[e * 64:(e + 1) * 64, j // 2, j % 2,
                              b * N:(b + 1) * N]
                    nc.vector.tensor_mul(dst, av[:, e, :], st["bcs"][0:64, e, :])
                del astate[p]

        def attn_items(u, psum_pool):
            g0 = 3 * u
            seq = [(g_scores, g0), (g_scores, g0 + 1), (g_denom, g0),
                   (g_scores, g0 + 2), (g_denom, g0 + 1), (g_av, g0),
                   (g_denom, g0 + 2), (g_av, g0 + 1), (g_av, g0 + 2)]
            return [(lambda fn=fn, g=g: fn(g, psum_pool)) for fn, g in seq]

        # In phase CD the attention psum tags (av/dn) are idle and their slots
        # are the same 1576B as a [128, CH] fp32 tile -> cycle through them
        # for a 6-deep GEMM psum rotation.
        _cd_tags = [[("mm", 3)]]
        _cd_i = [0]

        def cd_psum(psum_pool, name):
            tags = _cd_tags[0]
            tag, bufs = tags[_cd_i[0] % len(tags)]
            _cd_i[0] += 1
            return psum_pool.tile([128, CH], dt.float32, tag=tag, bufs=bufs,
                                  name=name)

        # ============ proj + residual1 + LN2 per-unit items ============
        # LN2 stats accumulate per-k as each residual column-block lands, so
        # the MLP of the next unit is not gated on a long serial chain.
        def projln2_items(u, psum_pool):
            cs = bass.ts(u, CH)
            items = []
            st = {}

            def proj_tile(d_i):
                def f():
                    pp = cd_psum(psum_pool, f"pj_{u}_{d_i}")
                    for kp in range(KP):
                        nc.tensor.matmul(pp[:], wp[:, kp, :, bass.ts(d_i, 128)],
                                         aoT[:, kp, :, cs], start=(kp == 0),
                                         stop=(kp == KP - 1), perf_mode=PM.DoubleRow)
                    if pb_any:
                        tmp = p_atmp.tile([128, CH], dt.float32, tag="rtmp", bufs=2)
                        nc.vector.tensor_scalar(tmp[:], pp[:], sp,
                                                t_pb[:, d_i:d_i + 1],
                                                op0=ALU.mult, op1=ALU.add)
                        nc.vector.tensor_add(xT[:, d_i, cs], tmp[:], xT[:, d_i, cs])
                    else:
                        with nc.allow_low_precision(reason="bf16 residual"):
                            nc.vector.scalar_tensor_tensor(xT[:, d_i, cs], pp[:],
                                                           sp, xT[:, d_i, cs],
                                                           op0=ALU.mult,
                                                           op1=ALU.add)
                    # fused LN2 stats for this k-tile
                    if d_i == 0:
                        st["mu"] = psum_pool.tile([1, CH], dt.float32, tag="sc",
                                                  bufs=2, name=f"mu2_{u}")
                        st["ms"] = psum_pool.tile([1, CH], dt.float32, tag="sc",
                                                  bufs=2, name=f"ms2_{u}")
                    x2 = p_atmp.tile([128, CH], dt.bfloat16, tag="ntmp", bufs=3)
                    nc.vector.tensor_mul(x2[:], xT[:, d_i, cs], xT[:, d_i, cs])
                    nc.tensor.matmul(st["mu"][:], ones_mu[:], xT[:, d_i, cs],
                                     start=(d_i == 0), stop=(d_i == KT - 1))
                    nc.tensor.matmul(st["ms"][:], ones_mu[:], x2[:],
                                     start=(d_i == 0), stop=(d_i == KT - 1))
                return f

            def rows():
                mu_ps, ms_ps = st["mu"], st["ms"]
                musq = p_rows.tile([1, CH], dt.float32, tag="musq")
                nc.scalar.square(musq[:], mu_ps[:])
                var = p_rows.tile([1, CH], dt.float32, tag="var")
                nc.vector.tensor_sub(var[:], ms_ps[:], musq[:])
                std = p_rows.tile([1, CH], dt.float32, tag="std")
                nc.scalar.activation(std[:], var[:], AF.Sqrt,
                                     bias=eps_t[0:1, 0:1], scale=1.0 / (HSC * HSC))
                a_f = p_rows.tile([1, CH], dt.float32, tag="af")
                nc.vector.reciprocal_approx_fast(a_f[:], std[:])
                a_b = p_rows.tile([1, CH], dt.bfloat16, tag="ab")
                nc.gpsimd.tensor_copy(a_b[:], a_f[:])
                b_b = p_rows.tile([1, CH], dt.bfloat16, tag="bb")
                with nc.allow_low_precision(reason="LN shift row bf16"):
                    nc.vector.scalar_tensor_tensor(b_b[:], mu_ps[:], -1.0, a_f[:],
                                                   op0=ALU.mult, op1=ALU.mult)
                bca = p_atmp.tile([128, CH], dt.bfloat16, tag="bca", bufs=2)
                nc.gpsimd.partition_broadcast(bca[:], a_b[:])
                bcb = p_atmp.tile([128, CH], dt.bfloat16, tag="bcb", bufs=2)
                nc.gpsimd.partition_broadcast(bcb[:], b_b[:])
                st["bca"], st["bcb"] = bca, bcb

            def apply(k0):
                for k in range(k0, k0 + 3):
                    tmp = p_atmp.tile([128, CH], dt.bfloat16, tag="ntmp", bufs=3)
                    nc.vector.tensor_mul(tmp[:], xT[:, k, cs], st["bca"][:])
                    nc.gpsimd.tensor_add(h1[:, k // 2, k % 2, cs], tmp[:],
                                         st["bcb"][:])

            for d_i in range(KT):
                items.append(proj_tile(d_i))
            items += [rows, lambda: apply(0), lambda: apply(3)]
            return items

        # ============ MLP per-unit items ============
        gdb = p_big.tile([128, 2, HP, 2, CH], dt.float8e4, tag="tg_v")

        def fc1_items(u, psum_pool):
            cs = bass.ts(u, CH)
            g = gdb[:, u % 2]
            items = []

            def fc1_tile(hh):
                def f():
                    pf = cd_psum(psum_pool, f"f1_{u}_{hh}")
                    for kp in range(KP):
                        nc.tensor.matmul(pf[:], w1[:, kp, :, bass.ts(hh, 128)],
                                         h1[:, kp, :, cs], start=(kp == 0),
                                         stop=(kp == KP - 1), perf_mode=PM.DoubleRow)
                    nc.scalar.activation(g[:, hh // 2, hh % 2, :], pf[:], AF.Gelu,
                                         bias=t_b1[:, hh:hh + 1], scale=s1)
                return f

            for hh in range(HT):
                items.append(fc1_tile(hh))
            return items

        def fc2_items(u, psum_pool):
            cs = bass.ts(u, CH)
            g = gdb[:, u % 2]
            items = []

            def fc2_tile(d_i):
                def f():
                    po = cd_psum(psum_pool, f"f2_{u}_{d_i}")
                    for hp in range(HP):
                        nc.tensor.matmul(po[:], w2[:, hp, :, bass.ts(d_i, 128)],
                                         g[:, hp, :, :], start=(hp == 0),
                                         stop=(hp == HP - 1), perf_mode=PM.DoubleRow)
                    y = p_y.tile([128, CH], dt.float32, tag="y", bufs=4,
                                 name=f"y_{u}_{d_i}")
                    if b2_any:
                        tmp = p_y.tile([128, CH], dt.float32, tag="ytmp", bufs=2)
                        nc.vector.tensor_scalar(tmp[:], po[:], s2,
                                                t_b2[:, d_i:d_i + 1],
                                                op0=ALU.mult, op1=ALU.add)
                        nc.vector.tensor_add(y[:], tmp[:], xT[:, d_i, cs])
                    else:
                        nc.vector.scalar_tensor_tensor(y[:], po[:], s2,
                                                       xT[:, d_i, cs],
                                                       op0=ALU.mult, op1=ALU.add)
                    nc.sync.dma_start(
                        d_yT.rearrange("(k p) t -> p k t", p=128)[:, d_i, cs],
                        y[:])
                return f

            for d_i in range(KT):
                items.append(fc2_tile(d_i))
            return items

        # ================= schedule =================
        # Phase AB: LN1(u)+QKV(u) || ATTN(u-1)
        p_aw = tc.alloc_tile_pool(name="paw", bufs=1)
        p_y = tc.alloc_tile_pool(name="py", bufs=1)
        psAB = tc.alloc_tile_pool(name="psAB", bufs=1, space="PSUM")
        w2 = p_big.tile([128, HP, 2, D], dt.float8e4, tag="tg_k")
        # LN1(u+1) runs a full step early so its serial row chain never
        # gates the QKV GEMMs of its own unit.
        _merge(ln_items(0, xT, h1, psAB, "sc"))
        for u in range(NU):
            streams = [qkv_items(u, psAB)]
            if u + 1 < NU:
                streams.append(ln_items(u + 1, xT, h1, psAB, "sc"))
            if 0 <= u - 1:
                streams.append(attn_items(u - 1, psAB))
            _merge(*streams)

        # Phase CD: the last attention unit overlaps PROJ+LN2(0) (which is
        # restricted to the "mm" psum tag while attention still owns av/dn),
        # then PROJ+LN2(u) || FC1(u-1) || FC2(u-2).
        nc.sync.dma_start(w2[:], d_w2.rearrange("(k p) m -> p k m", p=128)
                          .rearrange("p (a b) m -> p a b m", b=2))
        for u in range(NU + 2):
            if u == 1:
                _cd_tags[0] = [("mm", 3), ("mm", 3), ("mm", 3), ("av", 2),
                               ("av", 2), ("dn", 1)]
            streams = []
            if u == 0:
                streams.append(attn_items(NU - 1, psAB))
            if u < NU:
                streams.append(projln2_items(u, psAB))
            if 0 <= u - 2:
                streams.append(fc2_items(u - 2, psAB))
            if 0 <= u - 1 < NU:
                streams.append(fc1_items(u - 1, psAB))
            _merge(*streams)

        psAB.release()
        p_y.release()
        p_aw.release()
        p_vt.release()
        p_atmp.release()
        p_big.release()
        p_rows.release()
        p_const.release()

    nc.finalize()
    _NC_CACHE[flags] = nc
    return nc


def _quant_w(w):
    """fp8e4m3 quantize with power-of-2 upscale into the normal range."""
    amax = float(np.abs(w).max()) or 1.0
    S = int(np.floor(np.log2(224.0 / amax)))
    return (w * (2.0 ** S)).astype(f8), 2.0 ** (-S)


def _prep_host(inputs):
    f = np.float32
    x = np.asarray(inputs["x"], f)
    n1w, n1b = np.asarray(inputs["norm1_w"], f), np.asarray(inputs["norm1_b"], f)
    n2w, n2b = np.asarray(inputs["norm2_w"], f), np.asarray(inputs["norm2_b"], f)
    qkv_w = np.asarray(inputs["qkv_w"], f)
    q_bias, v_bias = np.asarray(inputs["q_bias"], f), np.asarray(inputs["v_bias"], f)
    rpb_table = np.asarray(inputs["rpb_table"], f)
    rel_index = np.asarray(inputs["rel_index"])
    proj_w, proj_b = np.asarray(inputs["proj_w"], f), np.asarray(inputs["proj_b"], f)
    g1, g2 = np.asarray(inputs["gamma1"], f), np.asarray(inputs["gamma2"], f)
    fc1_w, fc1_b = np.asarray(inputs["fc1_w"], f), np.asarray(inputs["fc1_b"], f)
    fc2_w, fc2_b = np.asarray(inputs["fc2_w"], f), np.asarray(inputs["fc2_b"], f)

    scale = DH ** -0.5
    Wq, Wk, Wv = qkv_w[0:D], qkv_w[D:2 * D], qkv_w[2 * D:3 * D]
    wqkvT = np.concatenate([(Wq * n1w).T, (Wk * n1w).T, (Wv * n1w).T], axis=1)
    wqkvT, rs_qkv = _quant_w(wqkvT)
    wqkvT = np.ascontiguousarray(wqkvT)
    qb = (HSC * scale * (Wq @ n1b + q_bias)).reshape(KT, 128).T.copy()
    kb = (HSC * (Wk @ n1b)).reshape(KT, 128).T.copy()
    vb_f = Wv @ n1b + v_bias
    wpT, rs_p = _quant_w((g1[:, None] * proj_w).T)
    wpT = np.ascontiguousarray(wpT)
    pb_f = g1 * proj_b
    pb = pb_f.reshape(KT, 128).T.copy()
    w1T, rs_1 = _quant_w((fc1_w * n2w).T)
    w1T = np.ascontiguousarray(w1T)
    b1 = (fc1_w @ n2b + fc1_b).reshape(HT, 128).T.copy()
    w2T, rs_2 = _quant_w((g2[:, None] * fc2_w).T)
    w2T = np.ascontiguousarray(w2T)
    b2_f = g2 * fc2_b
    b2 = b2_f.reshape(KT, 128).T.copy()

    # output scales: psum -> true value (row-replicated for [128,1] scalar APs)
    scales = np.tile(np.array([[scale * rs_qkv,         # sq (q,k scaled x16)
                                rs_qkv,                 # sk
                                rs_qkv / HSC,           # sv
                                rs_p / ASC,             # sp
                                rs_1 / HSC,             # s1 (gelu input)
                                rs_2,                   # s2
                                0.0, 0.0]], f), (128, 1))

    # rpb logit maps: rpbT[p, h, mt*N+n] = rpb[h, n, m]
    RPB = rpb_table[rel_index]            # [n, m, H]
    rpbT = np.zeros((128, H, MT * N), f)
    for mt in range(MT):
        msz = MSZ[mt]
        blk = RPB[:, mt * 128:mt * 128 + msz, :].transpose(1, 2, 0)  # [m, H, n]
        for h in range(H):
            rpbT[0:msz, h, mt * N:mt * N + N] = blk[:, h, :]
    rpbT = (rpbT * HSC * HSC).astype(f8)

    # v bias pre-scaled so it survives the sv = rs_qkv/HSC output scale
    vb = (vb_f * HSC / rs_qkv).reshape(1, D).astype(bf16)
    flags = (bool(np.any(vb_f)), bool(np.any(pb_f)), bool(np.any(b2_f)))
    shared = dict(wqkvT=wqkvT, wpT=wpT, w1T=w1T, w2T=w2T,
                  qb=np.ascontiguousarray(qb), kb=np.ascontiguousarray(kb),
                  vb=vb, pb=np.ascontiguousarray(pb),
                  b1=np.ascontiguousarray(b1), b2=np.ascontiguousarray(b2),
                  scales=scales, rpbT=rpbT,
                  eye=np.eye(128, dtype=f8))
    in_maps = []
    for core in range(NCORES):
        xs = x[core * BPC:(core + 1) * BPC]            # [BPC, N, D]
        m = dict(shared)
        m["xT"] = np.ascontiguousarray(xs.reshape(T, D).T.astype(bf16))
        in_maps.append(m)
    return in_maps, flags


def kernel(**inputs) -> np.ndarray:
    in_maps, flags = _prep_host(inputs)
    nc = _build_nc(flags)
    res = run_bass_kernel_spmd(nc, in_maps, core_ids=list(range(NCORES)))
    outs = []
    for core in range(NCORES):
        yT = res.results[core]["yT"]                   # [D, T]
        outs.append(np.asarray(yT, np.float32).T.reshape(BPC, N, D))
    return np.concatenate(outs, axis=0)


# revision 85
# speedup vs baseline: 1.0439x; 1.0439x over previous
"""Trainium2 Bass kernel for a ViT/Swin-style transformer block.

Strategy: data-parallel over batch (64 -> 8 per core), no collectives.
Feature-major on-device layout: activations [features(128-part, k-tiles),
tokens(free)].  Large GEMMs run in fp8e4m3 with DoubleRow perf mode
(256-deep contraction, 2x PE throughput) accumulating in fp32 PSUM;
attention score/AV matmuls are bf16.

The program is issued as two software-pipelined streams of per-unit work
items (unit = 2 batch elements = 394 token columns) so that the DVE/ACT
heavy attention overlaps the PE-heavy QKV GEMMs, and the DVE-heavy
proj+LN2 overlaps the PE/ACT-heavy MLP:
    LN1(u)+QKV(u)  ||  ATTN(u-1)
    PROJ+LN2(u)    ||  MLP(u-1)

Quantization scales (folded, compensated in the psum->sbuf cast):
  weights  *2^Sw  (per-tensor, host-chosen to fill e4m3 normal range)
  h1/h2    *16    (LN outputs; folded into the Sqrt(var) stage for free)
  aoT      *32    (attention out; folded into the softmax reciprocal)

Softmax: scores + rpb (PE identity-matmul accumulate) -> exp (ACT) ->
denominator via ones-column matmul (PE) -> reciprocal (DVE) ->
partition_broadcast (Pool) -> folded into the AV-psum -> aoT fp8 cast.
"""

import numpy as np
import ml_dtypes
from contextlib import ExitStack

import concourse.bacc as bacc
import concourse.bass as bass
import concourse.mybir as mybir
import concourse.tile as tile
from concourse.bass_utils import run_bass_kernel_spmd

bf16 = ml_dtypes.bfloat16
dt = mybir.dt
f8 = mybir.dt.np(dt.float8e4)
AF = mybir.ActivationFunctionType
ALU = mybir.AluOpType
PM = mybir.MatmulPerfMode

# ---- problem dims (hardcoded) ----
B, N, D, H, DH, HID = 64, 197, 768, 12, 64, 3072
NCORES = 8
BPC = B // NCORES          # 8 batch elements per core
T = BPC * N                # 1576 token-columns per core
KT = D // 128              # 6 feature k-tiles
KP = KT // 2               # 3 DoubleRow k-pairs
HT = HID // 128            # 24 hidden tiles
HP = HT // 2               # 12 DoubleRow hidden pairs
NU = 4                     # units (2 batch elements each)
CH = T // NU               # 394 token-cols per unit
MT = 2                     # m-tiles per batch element (128 + 69)
MSZ = [128, N - 128]       # [128, 69]
EPS = 1e-5
HSC = 16.0                 # LN-output fp8 scale
ASC = 32.0                 # attention-output fp8 scale

_NC_CACHE = {}


def _merge(*streams):
    """Proportional round-robin: issue items from all streams interleaved
    so every stream finishes at the same fractional position."""
    counts = [len(s) for s in streams]
    mx = max(counts) if counts else 0
    for t in range(mx):
        for s, cnt in zip(streams, counts):
            for item in s[t * cnt // mx:(t + 1) * cnt // mx]:
                item()


def _build_nc(flags):
    if flags in _NC_CACHE:
        return _NC_CACHE[flags]
    vb_any, pb_any, b2_any = flags
    nc = bacc.Bacc(None, target_bir_lowering=False)

    # ---- DRAM I/O ----
    d_xT = nc.dram_tensor("xT", [D, T], dt.bfloat16, kind="ExternalInput")
    d_wqkv = nc.dram_tensor("wqkvT", [D, 3 * D], dt.float8e4, kind="ExternalInput")
    d_wp = nc.dram_tensor("wpT", [D, D], dt.float8e4, kind="ExternalInput")
    d_w1 = nc.dram_tensor("w1T", [D, HID], dt.float8e4, kind="ExternalInput")
    d_w2 = nc.dram_tensor("w2T", [HID, D], dt.float8e4, kind="ExternalInput")
    d_qb = nc.dram_tensor("qb", [128, KT], dt.float32, kind="ExternalInput")
    d_kb = nc.dram_tensor("kb", [128, KT], dt.float32, kind="ExternalInput")
    d_vb = nc.dram_tensor("vb", [1, D], dt.bfloat16, kind="ExternalInput")
    d_pb = nc.dram_tensor("pb", [128, KT], dt.float32, kind="ExternalInput")
    d_b1 = nc.dram_tensor("b1", [128, HT], dt.float32, kind="ExternalInput")
    d_b2 = nc.dram_tensor("b2", [128, KT], dt.float32, kind="ExternalInput")
    d_sc = nc.dram_tensor("scales", [128, 8], dt.float32, kind="ExternalInput")
    d_eye = nc.dram_tensor("eye", [128, 128], dt.float8e4, kind="ExternalInput")
    d_rpb = nc.dram_tensor("rpbT", [128, H, MT * N], dt.float8e4, kind="ExternalInput")
    d_yT = nc.dram_tensor("yT", [D, T], dt.float32, kind="ExternalOutput")

    with ExitStack() as ctx:
        tc = ctx.enter_context(tile.TileContext(nc))

        p_const = tc.alloc_tile_pool(name="const", bufs=1)
        p_rows = tc.alloc_tile_pool(name="prows", bufs=2)
        p_big = tc.alloc_tile_pool(name="pbig", bufs=1)

        # constants
        ones_mu = p_const.tile([128, 1], dt.bfloat16)      # 1/768 column
        ones_r = p_const.tile([1, 128], dt.bfloat16)       # 1.0 row
        eps_t = p_const.tile([1, 1], dt.float32)
        nc.vector.memset(ones_mu[:], 1.0 / D)
        nc.vector.memset(ones_r[:], 1.0)
        nc.vector.memset(eps_t[:], EPS / (HSC * HSC))
        t_eye = p_const.tile([128, 128], dt.float8e4)
        nc.sync.dma_start(t_eye[:], d_eye[:])
        ones_w = p_const.tile([1, 512], dt.bfloat16)
        nc.vector.memset(ones_w[:], 1.0)
        t_qb = p_const.tile([128, KT], dt.float32)
        t_kb = p_const.tile([128, KT], dt.float32)
        t_vb = p_const.tile([1, D], dt.bfloat16)
        t_pb = p_const.tile([128, KT], dt.float32)
        t_b1 = p_const.tile([128, HT], dt.float32)
        t_b2 = p_const.tile([128, KT], dt.float32)
        t_sc = p_const.tile([128, 8], dt.float32)
        for t_, d_ in [(t_qb, d_qb), (t_kb, d_kb), (t_vb, d_vb),
                       (t_pb, d_pb), (t_b1, d_b1), (t_b2, d_b2), (t_sc, d_sc)]:
            nc.sync.dma_start(t_[:], d_[:])
        sq, sk, sv, sp, s1, s2 = (t_sc[:, i:i + 1] for i in range(6))

        # ---- long-lived tiles ----
        # x lives in bf16: LN inputs read it directly and the residual
        # updates round through bf16 (error ~0.4% of |x|, within tolerance)
        xT = p_big.tile([128, KT, T], dt.bfloat16, tag="tg_x")
        xTr = d_xT.rearrange("(k p) t -> p k t", p=128)
        for c in range(NU):   # chunk-major so LN1(0) starts after 1st DMA
            cs = bass.ts(c, CH)
            nc.gpsimd.dma_start(xT[:, :, cs], xTr[:, :, cs])

        wqkv = p_big.tile([128, KP, 2, 3 * D], dt.float8e4, tag="tg_wqkv")
        nc.sync.dma_start(wqkv[:], d_wqkv.rearrange("(k p) m -> p k m", p=128)
                          .rearrange("p (a b) m -> p a b m", b=2))
        wp = p_big.tile([128, KP, 2, D], dt.float8e4, tag="tg_wp")
        nc.sync.dma_start(wp[:], d_wp.rearrange("(k p) m -> p k m", p=128)
                          .rearrange("p (a b) m -> p a b m", b=2))
        rpb = p_big.tile([128, H, MT * N], dt.float8e4, tag="tg_rpb")
        nc.sync.dma_start(rpb[:], d_rpb[:])

        w1 = p_big.tile([128, KP, 2, HID], dt.float8e4, tag="tg_w1")
        nc.sync.dma_start(w1[:], d_w1.rearrange("(k p) m -> p k m", p=128)
                          .rearrange("p (a b) m -> p a b m", b=2))

        h1 = p_big.tile([128, KP, 2, T], dt.float8e4, tag="tg_h")
        qT = p_big.tile([128, KT, T], dt.float8e4, tag="tg_q")
        kTt = p_big.tile([128, KT, T], dt.float8e4, tag="tg_k")
        # token-major V: vtokT[token, b, mt, ktile, feat] (feat = 2 heads x 64)
        vtokT = p_big.tile([128, BPC, MT, KT, 128], dt.bfloat16, tag="tg_v")
        aoT = p_big.tile([128, KP, 2, T], dt.float8e4, tag="tg_ao")

        p_atmp = tc.alloc_tile_pool(name="patmp", bufs=1)
        p_vt = tc.alloc_tile_pool(name="pvt", bufs=1)

        # ============ LayerNorm per-chunk items (feature-major) ============
        # dst_f8[:, kp, e, cs] = HSC * (x - mu)/sigma
        def ln_items(c, src_bf, dst_f8, psum_pool, stat_tag):
            cs = bass.ts(c, CH)
            st = {}

            def stats():
                x2 = p_atmp.tile([128, KT, CH], dt.bfloat16, tag="x2", bufs=2)
                nc.vector.tensor_mul(x2[:], src_bf[:, :, cs], src_bf[:, :, cs])
                mu_ps = psum_pool.tile([1, CH], dt.float32, tag=stat_tag, bufs=2)
                ms_ps = psum_pool.tile([1, CH], dt.float32, tag=stat_tag, bufs=2)
                for k in range(KT):
                    nc.tensor.matmul(mu_ps[:], ones_mu[:], src_bf[:, k, cs],
                                     start=(k == 0), stop=(k == KT - 1))
                for k in range(KT):
                    nc.tensor.matmul(ms_ps[:], ones_mu[:], x2[:, k, :],
                                     start=(k == 0), stop=(k == KT - 1))
                st["mu"], st["ms"] = mu_ps, ms_ps

            def rows():
                mu_ps, ms_ps = st["mu"], st["ms"]
                musq = p_rows.tile([1, CH], dt.float32, tag="musq")
                nc.scalar.square(musq[:], mu_ps[:])
                var = p_rows.tile([1, CH], dt.float32, tag="var")
                nc.vector.tensor_sub(var[:], ms_ps[:], musq[:])
                std = p_rows.tile([1, CH], dt.float32, tag="std")
                nc.scalar.activation(std[:], var[:], AF.Sqrt,
                                     bias=eps_t[0:1, 0:1], scale=1.0 / (HSC * HSC))
                a_f = p_rows.tile([1, CH], dt.float32, tag="af")
                nc.vector.reciprocal_approx_fast(a_f[:], std[:])     # HSC/sigma
                a_b = p_rows.tile([1, CH], dt.bfloat16, tag="ab")
                nc.gpsimd.tensor_copy(a_b[:], a_f[:])
                b_b = p_rows.tile([1, CH], dt.bfloat16, tag="bb")
                with nc.allow_low_precision(reason="LN shift row bf16"):
                    nc.vector.scalar_tensor_tensor(b_b[:], mu_ps[:], -1.0, a_f[:],
                                                   op0=ALU.mult, op1=ALU.mult)
                bca = p_atmp.tile([128, CH], dt.bfloat16, tag="bca", bufs=2)
                nc.gpsimd.partition_broadcast(bca[:], a_b[:])
                bcb = p_atmp.tile([128, CH], dt.bfloat16, tag="bcb", bufs=2)
                nc.gpsimd.partition_broadcast(bcb[:], b_b[:])
                st["bca"], st["bcb"] = bca, bcb

            def apply(k0):
                for k in range(k0, k0 + 3):
                    tmp = p_atmp.tile([128, CH], dt.bfloat16, tag="ntmp", bufs=3)
                    nc.vector.tensor_mul(tmp[:], src_bf[:, k, cs], st["bca"][:])
                    eng = nc.vector if k % 2 == 0 else nc.gpsimd
                    eng.tensor_add(dst_f8[:, k // 2, k % 2, cs], tmp[:],
                                   st["bcb"][:])

            return [stats, rows, lambda: apply(0), lambda: apply(3)]

        # ============ QKV per-unit items ============
        def qkv_items(u, psum_pool):
            cs = bass.ts(u, CH)
            items = []

            def qk_tile(d_i, is_q):
                def f():
                    off = 0 if is_q else D
                    ps = psum_pool.tile([128, CH], dt.float32, tag="mm", bufs=3)
                    for kp in range(KP):
                        nc.tensor.matmul(ps[:], wqkv[:, kp, :, off + d_i * 128:
                                                     off + (d_i + 1) * 128],
                                         h1[:, kp, :, cs], start=(kp == 0),
                                         stop=(kp == KP - 1), perf_mode=PM.DoubleRow)
                    if is_q:
                        nc.scalar.activation(qT[:, d_i, cs], ps[:], AF.Identity,
                                             bias=t_qb[:, d_i:d_i + 1], scale=sq)
                    else:
                        nc.vector.tensor_scalar(kTt[:, d_i, cs], ps[:], sk,
                                                t_kb[:, d_i:d_i + 1],
                                                op0=ALU.mult, op1=ALU.add)
                return f

            def v_tile(d_i, vT):
                def f():
                    pv = psum_pool.tile([128, CH], dt.float32, tag="mm", bufs=3)
                    if vb_any:
                        nc.tensor.matmul(pv[:], t_vb[:, d_i * 128:(d_i + 1) * 128],
                                         ones_w[0:1, 0:CH], start=True, stop=False,
                                         skip_group_check=True)
                    for kp in range(KP):
                        nc.tensor.matmul(pv[:], wqkv[:, kp, :,
                                                     2 * D + d_i * 128:
                                                     2 * D + (d_i + 1) * 128],
                                         h1[:, kp, :, cs],
                                         start=(kp == 0 and not vb_any),
                                         stop=(kp == KP - 1), perf_mode=PM.DoubleRow,
                                         skip_group_check=vb_any)
                    if d_i % 2 == 0:
                        nc.scalar.activation(vT[:, d_i, 0:CH], pv[:], AF.Identity,
                                             scale=sv)
                    else:
                        nc.vector.tensor_scalar(vT[:, d_i, 0:CH], pv[:], sv, 0.0,
                                                op0=ALU.mult, op1=ALU.add)
                    # transpose this k-tile for both (b, mt) of the unit
                    for bl in range(2):
                        for mt in range(MT):
                            off = bl * N + mt * 128
                            nc.sync.dma_start_transpose(
                                vtokT[:, 2 * u + bl, mt, d_i, :],
                                vT[:, d_i, off:off + 128])
                return f

            vT = p_vt.tile([128, KT, 512], dt.bfloat16, tag="vt", bufs=2,
                           name=f"vT_{u}")
            for d_i in range(KT):
                items.append(qk_tile(d_i, True))
                items.append(qk_tile(d_i, False))
                items.append(v_tile(d_i, vT))
            return items

        # ============ attention per-unit items ============
        PAIRS = [(b, j) for b in range(BPC) for j in range(H // 2)]
        GRP = 4
        GROUPS = [PAIRS[g * GRP:(g + 1) * GRP] for g in range(len(PAIRS) // GRP)]
        astate = {}

        def g_scores(g, psum_pool):
            for p in GROUPS[g]:
                b, j = p
                ts_n = slice(b * N, (b + 1) * N)
                p_bf = p_aw.tile([128, 2, MT * N], dt.bfloat16, tag="pbf", bufs=7,
                                 name=f"pbf_{b}_{j}")
                for e in range(2):
                    hp = e * 64
                    sc = psum_pool.tile([128, MT * N], dt.float32, tag="sc", bufs=2,
                                        name=f"sc_{b}_{j}_{e}")
                    for mt in range(MT):
                        msz = MSZ[mt]
                        m0 = b * N + mt * 128
                        nc.tensor.matmul(sc[0:msz, bass.ts(mt, N)],
                                         kTt[hp:hp + 64, j, m0:m0 + msz],
                                         qT[hp:hp + 64, j, ts_n],
                                         start=True, stop=False,
                                         skip_group_check=True)
                    nc.tensor.matmul(sc[:], t_eye[:], rpb[:, 2 * j + e, :],
                                     start=False, stop=True, skip_group_check=True)
                    nc.scalar.activation(p_bf[:, e, :], sc[:], AF.Exp,
                                         scale=1.0 / (HSC * HSC))
                astate[p] = {"p_bf": p_bf}

        def g_denom(g, psum_pool):
            for p in GROUPS[g]:
                b, j = p
                pb = astate[p]["p_bf"]
                dn = psum_pool.tile([1, 2, N], dt.float32, tag="dn", bufs=1,
                                    name=f"dn_{b}_{j}")
                for mt in range(MT):
                    msz = MSZ[mt]
                    nc.tensor.matmul(dn[:], ones_mu[0:msz, 0:1],
                                     pb[0:msz, :, mt * N:(mt + 1) * N],
                                     start=(mt == 0), stop=(mt == MT - 1))
                rc = p_aw.tile([1, 2 * N], dt.float32, tag="rc", bufs=2,
                               name=f"rc_{b}_{j}")
                nc.vector.reciprocal_approx_fast(
                    rc[:], dn[:].rearrange("a e n -> a (e n)"))
                rcb = p_aw.tile([1, 2 * N], dt.bfloat16, tag="rcb", bufs=2,
                                name=f"rcb_{b}_{j}")
                with nc.allow_low_precision(reason="softmax recip bf16"):
                    nc.gpsimd.tensor_scalar(rcb[:], rc[:], ASC / D, 0.0,
                                            op0=ALU.mult, op1=ALU.add)
                bcs = p_aw.tile([128, 2, N], dt.bfloat16, tag="bcs", bufs=4,
                                name=f"bcs_{b}_{j}")
                nc.gpsimd.partition_broadcast(
                    bcs[:].rearrange("p e n -> p (e n)"), rcb[:])
                astate[p]["bcs"] = bcs

        def g_av(g, psum_pool):
            for p in GROUPS[g]:
                b, j = p
                st = astate[p]
                av = psum_pool.tile([64, 2, N], dt.float32, tag="av", bufs=2,
                                    name=f"av_{b}_{j}")
                for e in range(2):
                    for mt in range(MT):
                        nc.tensor.matmul(av[:, e, :],
                                         vtokT[0:MSZ[mt], b, mt, j,
                                               e * 64:(e + 1) * 64],
                                         st["p_bf"][0:MSZ[mt], e,
                                                    mt * N:(mt + 1) * N],
                                         start=(mt == 0), stop=(mt == MT - 1))
                for e in range(2):
                    dst = aoT[e * 64:(e + 1) * 64, j // 2, j % 2,
                              b * N:(b + 1) * N]
                    nc.vector.tensor_mul(dst, av[:, e, :], st["bcs"][0:64, e, :])
                del astate[p]

        def attn_items(u, psum_pool):
            g0 = 3 * u
            seq = [(g_scores, g0), (g_scores, g0 + 1), (g_denom, g0),
                   (g_scores, g0 + 2), (g_denom, g0 + 1), (g_av, g0),
                   (g_denom, g0 + 2), (g_av, g0 + 1), (g_av, g0 + 2)]
            return [(lambda fn=fn, g=g: fn(g, psum_pool)) for fn, g in seq]

        # In phase CD the attention psum tags (av/dn) are idle and their slots
        # are the same 1576B as a [128, CH] fp32 tile -> cycle through them
        # for a 6-deep GEMM psum rotation.
        _cd_tags = [[("mm", 3)]]
        _cd_i = [0]

        def cd_psum(psum_pool, name):
            tags = _cd_tags[0]
            tag, bufs = tags[_cd_i[0] % len(tags)]
            _cd_i[0] += 1
            return psum_pool.tile([128, CH], dt.float32, tag=tag, bufs=bufs,
                                  name=name)

        # ============ proj + residual1 + LN2 per-unit items ============
        # LN2 stats accumulate per-k as each residual column-block lands, so
        # the MLP of the next unit is not gated on a long serial chain.
        def projln2_items(u, psum_pool):
            cs = bass.ts(u, CH)
            items = []
            st = {}

            def proj_tile(d_i):
                def f():
                    pp = cd_psum(psum_pool, f"pj_{u}_{d_i}")
                    for kp in range(KP):
                        nc.tensor.matmul(pp[:], wp[:, kp, :, bass.ts(d_i, 128)],
                                         aoT[:, kp, :, cs], start=(kp == 0),
                                         stop=(kp == KP - 1), perf_mode=PM.DoubleRow)
                    if pb_any:
                        tmp = p_atmp.tile([128, CH], dt.float32, tag="rtmp", bufs=2)
                        nc.vector.tensor_scalar(tmp[:], pp[:], sp,
                                                t_pb[:, d_i:d_i + 1],
                                                op0=ALU.mult, op1=ALU.add)
                        nc.vector.tensor_add(xT[:, d_i, cs], tmp[:], xT[:, d_i, cs])
                    else:
                        with nc.allow_low_precision(reason="bf16 residual"):
                            nc.vector.scalar_tensor_tensor(xT[:, d_i, cs], pp[:],
                                                           sp, xT[:, d_i, cs],
                                                           op0=ALU.mult,
                                                           op1=ALU.add)
                    # fused LN2 stats for this k-tile
                    if d_i == 0:
                        st["mu"] = psum_pool.tile([1, CH], dt.float32, tag="sc",
                                                  bufs=2, name=f"mu2_{u}")
                        st["ms"] = psum_pool.tile([1, CH], dt.float32, tag="sc",
                                                  bufs=2, name=f"ms2_{u}")
                    x2 = p_atmp.tile([128, CH], dt.bfloat16, tag="ntmp", bufs=3)
                    nc.vector.tensor_mul(x2[:], xT[:, d_i, cs], xT[:, d_i, cs])
                    nc.tensor.matmul(st["mu"][:], ones_mu[:], xT[:, d_i, cs],
                                     start=(d_i == 0), stop=(d_i == KT - 1))
                    nc.tensor.matmul(st["ms"][:], ones_mu[:], x2[:],
                                     start=(d_i == 0), stop=(d_i == KT - 1))
                return f

            def rows():
                mu_ps, ms_ps = st["mu"], st["ms"]
                musq = p_rows.tile([1, CH], dt.float32, tag="musq")
                nc.scalar.square(musq[:], mu_ps[:])
                var = p_rows.tile([1, CH], dt.float32, tag="var")
                nc.vector.tensor_sub(var[:], ms_ps[:], musq[:])
                std = p_rows.tile([1, CH], dt.float32, tag="std")
                nc.scalar.activation(std[:], var[:], AF.Sqrt,
                                     bias=eps_t[0:1, 0:1], scale=1.0 / (HSC * HSC))
                a_f = p_rows.tile([1, CH], dt.float32, tag="af")
                nc.vector.reciprocal_approx_fast(a_f[:], std[:])
                a_b = p_rows.tile([1, CH], dt.bfloat16, tag="ab")
                nc.gpsimd.tensor_copy(a_b[:], a_f[:])
                b_b = p_rows.tile([1, CH], dt.bfloat16, tag="bb")
                with nc.allow_low_precision(reason="LN shift row bf16"):
                    nc.vector.scalar_tensor_tensor(b_b[:], mu_ps[:], -1.0, a_f[:],
                                                   op0=ALU.mult, op1=ALU.mult)
                bca = p_atmp.tile([128, CH], dt.bfloat16, tag="bca", bufs=2)
                nc.gpsimd.partition_broadcast(bca[:], a_b[:])
                bcb = p_atmp.tile([128, CH], dt.bfloat16, tag="bcb", bufs=2)
                nc.gpsimd.partition_broadcast(bcb[:], b_b[:])
                st["bca"], st["bcb"] = bca, bcb

            def apply(k0):
                for k in range(k0, k0 + 3):
                    tmp = p_atmp.tile([128, CH], dt.bfloat16, tag="ntmp", bufs=3)
                    nc.vector.tensor_mul(tmp[:], xT[:, k, cs], st["bca"][:])
                    eng = nc.vector if k % 2 == 0 else nc.gpsimd
                    eng.tensor_add(h1[:, k // 2, k % 2, cs], tmp[:], st["bcb"][:])

            for d_i in range(KT):
                items.append(proj_tile(d_i))
            items += [rows, lambda: apply(0), lambda: apply(3)]
            return items

        # ============ MLP per-unit items ============
        gdb = p_big.tile([128, 2, HP, 2, CH], dt.float8e4, tag="tg_v")

        def fc1_items(u, psum_pool):
            cs = bass.ts(u, CH)
            g = gdb[:, u % 2]
            items = []

            def fc1_tile(hh):
                def f():
                    pf = cd_psum(psum_pool, f"f1_{u}_{hh}")
                    for kp in range(KP):
                        nc.tensor.matmul(pf[:], w1[:, kp, :, bass.ts(hh, 128)],
                                         h1[:, kp, :, cs], start=(kp == 0),
                                         stop=(kp == KP - 1), perf_mode=PM.DoubleRow)
                    nc.scalar.activation(g[:, hh // 2, hh % 2, :], pf[:], AF.Gelu,
                                         bias=t_b1[:, hh:hh + 1], scale=s1)
                return f

            for hh in range(HT):
                items.append(fc1_tile(hh))
            return items

        def fc2_items(u, psum_pool):
            cs = bass.ts(u, CH)
            g = gdb[:, u % 2]
            items = []

            def fc2_tile(d_i):
                def f():
                    po = cd_psum(psum_pool, f"f2_{u}_{d_i}")
                    for hp in range(HP):
                        nc.tensor.matmul(po[:], w2[:, hp, :, bass.ts(d_i, 128)],
                                         g[:, hp, :, :], start=(hp == 0),
                                         stop=(hp == HP - 1), perf_mode=PM.DoubleRow)
                    y = p_y.tile([128, CH], dt.float32, tag="y", bufs=4,
                                 name=f"y_{u}_{d_i}")
                    if b2_any:
                        tmp = p_y.tile([128, CH], dt.float32, tag="ytmp", bufs=2)
                        nc.vector.tensor_scalar(tmp[:], po[:], s2,
                                                t_b2[:, d_i:d_i + 1],
                                                op0=ALU.mult, op1=ALU.add)
                        nc.vector.tensor_add(y[:], tmp[:], xT[:, d_i, cs])
                    else:
                        nc.vector.scalar_tensor_tensor(y[:], po[:], s2,
                                                       xT[:, d_i, cs],
                                                       op0=ALU.mult, op1=ALU.add)
                    nc.sync.dma_start(
                        d_yT.rearrange("(k p) t -> p k t", p=128)[:, d_i, cs],
                        y[:])
                return f

            for d_i in range(KT):
                items.append(fc2_tile(d_i))
            return items

        # ================= schedule =================
        # Phase AB: LN1(u)+QKV(u) || ATTN(u-1)
        p_aw = tc.alloc_tile_pool(name="paw", bufs=1)
        p_y = tc.alloc_tile_pool(name="py", bufs=1)
        psAB = tc.alloc_tile_pool(name="psAB", bufs=1, space="PSUM")
        w2 = p_big.tile([128, HP, 2, D], dt.float8e4, tag="tg_k")
        # LN1(u+1) runs a full step early so its serial row chain never
        # gates the QKV GEMMs of its own unit.
        _merge(ln_items(0, xT, h1, psAB, "sc"))
        for u in range(NU):
            streams = [qkv_items(u, psAB)]
            if u + 1 < NU:
                streams.append(ln_items(u + 1, xT, h1, psAB, "sc"))
            if 0 <= u - 1:
                streams.append(attn_items(u - 1, psAB))
            _merge(*streams)

        # Phase CD: the last attention unit overlaps PROJ+LN2(0) (which is
        # restricted to the "mm" psum tag while attention still owns av/dn),
        # then PROJ+LN2(u) || FC1(u-1) || FC2(u-2).
        nc.sync.dma_start(w2[:], d_w2.rearrange("(k p) m -> p k m", p=128)
                          .rearrange("p (a b) m -> p a b m", b=2))
        for u in range(NU + 2):
            if u == 1:
                _cd_tags[0] = [("mm", 3), ("mm", 3), ("mm", 3), ("av", 2),
                               ("av", 2), ("dn", 1)]
            streams = []
            if u == 0:
                streams.append(attn_items(NU - 1, psAB))
            if u < NU:
                streams.append(projln2_items(u, psAB))
            if 0 <= u - 1 < NU:
                streams.append(fc1_items(u - 1, psAB))
            if 0 <= u - 2:
                streams.append(fc2_items(u - 2, psAB))
            _merge(*streams)

        psAB.release()
        p_y.release()
        p_aw.release()
        p_vt.release()
        p_atmp.release()
        p_big.release()
        p_rows.release()
        p_const.release()

    nc.finalize()
    _NC_CACHE[flags] = nc
    return nc


def _quant_w(w):
    """fp8e4m3 quantize with power-of-2 upscale into the normal range."""
    amax = float(np.abs(w).max()) or 1.0
    S = int(np.floor(np.log2(224.0 / amax)))
    return (w * (2.0 ** S)).astype(f8), 2.0 ** (-S)


def _prep_host(inputs):
    f = np.float32
    x = np.asarray(inputs["x"], f)
    n1w, n1b = np.asarray(inputs["norm1_w"], f), np.asarray(inputs["norm1_b"], f)
    n2w, n2b = np.asarray(inputs["norm2_w"], f), np.asarray(inputs["norm2_b"], f)
    qkv_w = np.asarray(inputs["qkv_w"], f)
    q_bias, v_bias = np.asarray(inputs["q_bias"], f), np.asarray(inputs["v_bias"], f)
    rpb_table = np.asarray(inputs["rpb_table"], f)
    rel_index = np.asarray(inputs["rel_index"])
    proj_w, proj_b = np.asarray(inputs["proj_w"], f), np.asarray(inputs["proj_b"], f)
    g1, g2 = np.asarray(inputs["gamma1"], f), np.asarray(inputs["gamma2"], f)
    fc1_w, fc1_b = np.asarray(inputs["fc1_w"], f), np.asarray(inputs["fc1_b"], f)
    fc2_w, fc2_b = np.asarray(inputs["fc2_w"], f), np.asarray(inputs["fc2_b"], f)

    scale = DH ** -0.5
    Wq, Wk, Wv = qkv_w[0:D], qkv_w[D:2 * D], qkv_w[2 * D:3 * D]
    wqkvT = np.concatenate([(Wq * n1w).T, (Wk * n1w).T, (Wv * n1w).T], axis=1)
    wqkvT, rs_qkv = _quant_w(wqkvT)
    wqkvT = np.ascontiguousarray(wqkvT)
    qb = (HSC * scale * (Wq @ n1b + q_bias)).reshape(KT, 128).T.copy()
    kb = (HSC * (Wk @ n1b)).reshape(KT, 128).T.copy()
    vb_f = Wv @ n1b + v_bias
    wpT, rs_p = _quant_w((g1[:, None] * proj_w).T)
    wpT = np.ascontiguousarray(wpT)
    pb_f = g1 * proj_b
    pb = pb_f.reshape(KT, 128).T.copy()
    w1T, rs_1 = _quant_w((fc1_w * n2w).T)
    w1T = np.ascontiguousarray(w1T)
    b1 = (fc1_w @ n2b + fc1_b).reshape(HT, 128).T.copy()
    w2T, rs_2 = _quant_w((g2[:, None] * fc2_w).T)
    w2T = np.ascontiguousarray(w2T)
    b2_f = g2 * fc2_b
    b2 = b2_f.reshape(KT, 128).T.copy()

    # output scales: psum -> true value (row-replicated for [128,1] scalar APs)
    scales = np.tile(np.array([[scale * rs_qkv,         # sq (q,k scaled x16)
                                rs_qkv,                 # sk
                                rs_qkv / HSC,           # sv
                                rs_p / ASC,             # sp
                                rs_1 / HSC,             # s1 (gelu input)
                                rs_2,                   # s2
                                0.0, 0.0]], f), (128, 1))

    # rpb logit maps: rpbT[p, h, mt*N+n] = rpb[h, n, m]
    RPB = rpb_table[rel_index]            # [n, m, H]
    rpbT = np.zeros((128, H, MT * N), f)
    for mt in range(MT):
        msz = MSZ[mt]
        blk = RPB[:, mt * 128:mt * 128 + msz, :].transpose(1, 2, 0)  # [m, H, n]
        for h in range(H):
            rpbT[0:msz, h, mt * N:mt * N + N] = blk[:, h, :]
    rpbT = (rpbT * HSC * HSC).astype(f8)

    # v bias pre-scaled so it survives the sv = rs_qkv/HSC output scale
    vb = (vb_f * HSC / rs_qkv).reshape(1, D).astype(bf16)
    flags = (bool(np.any(vb_f)), bool(np.any(pb_f)), bool(np.any(b2_f)))
    shared = dict(wqkvT=wqkvT, wpT=wpT, w1T=w1T, w2T=w2T,
                  qb=np.ascontiguousarray(qb), kb=np.ascontiguousarray(kb),
                  vb=vb, pb=np.ascontiguousarray(pb),
                  b1=np.ascontiguousarray(b1), b2=np.ascontiguousarray(b2),
                  scales=scales, rpbT=rpbT,
                  eye=np.eye(128, dtype=f8))
    in_maps = []
    for core in range(NCORES):
        xs = x[core * BPC:(core + 1) * BPC]            # [BPC, N, D]
        m = dict(shared)
        m["xT"] = np.ascontiguousarray(xs.reshape(T, D).T.astype(bf16))
        in_maps.append(m)
    return in_maps, flags


def kernel(**inputs) -> np.ndarray:
    in_maps, flags = _prep_host(inputs)
    nc = _build_nc(flags)
    res = run_bass_kernel_spmd(nc, in_maps, core_ids=list(range(NCORES)))
    outs = []
    for core in range(NCORES):
        yT = res.results[core]["yT"]                   # [D, T]
        outs.append(np.asarray(yT, np.float32).T.reshape(BPC, N, D))
    return np.concatenate(outs, axis=0)
